# revision 5
# baseline (speedup 1.0000x reference)
"""Multi-head causal attention (B=2, T=2048, C=1024, H=16, D=64) on 8 TRN2 cores.

Sharding: 2 heads per core (tensor-parallel over H). x is replicated (passed
pre-transposed as x^T so the contraction dim lands on SBUF partitions). Each
core computes y[:, :, 2c*64:(2c+2)*64]; host concatenates along channels.

Per-core dataflow (all matmuls float32r = full PE rate at N>=256):
  1. Projections, W stationary: Q^T,K^T,V^T in [dd=2*64, t] layout
     (both heads stacked on partitions). Scores scale 1/sqrt(C) folded into Wq.
  2. V^T PE-transposed to V[s, d] per head, with a ones column appended
     (V_aug[s, 65]) so the AV matmul also produces softmax sums for free.
  3. Scores S^T[s, t] = K^T(stationary) x Q^T(moving), causal handled at
     block granularity; diagonal blocks get a triangular -1e10 mask add.
  4. exp on ScalarE (no max-subtraction needed: |scores| <= ~1), PSUM->SBUF,
     fully-masked sub-blocks zeroed on GpSimd.
  5. AV: V_aug stationary, E^T moving, accumulated over s-blocks in PSUM ->
     out^T[65, t] (row 64 = sums).
  6. PE-transpose to [t, 65], reciprocal + per-partition scalar multiply,
     DMA out.
"""

import numpy as np

import concourse.mybir as mybir
import concourse.tile as tile
from concourse import bacc
from concourse.masks import make_identity

B, T, C, H, D = 2, 2048, 1024, 16, 64
HPC = 2          # heads per core
NCORES = 8
TT = 512         # t-tile (moving free dim)
SB = 128         # s-block (scores stationary free dim)
NCH = C // 128   # contraction chunks for projections
GS = 3           # s-blocks per exp group
F32 = mybir.dt.float32
F32R = mybir.dt.float32r
MASK_VAL = -1e10


def build_nc(t_len=T, batches=B):
    nj = t_len // TT
    nc = bacc.Bacc("TRN2", target_bir_lowering=False, debug=False)
    xt = nc.dram_tensor("xt", [batches, C, t_len], F32R, kind="ExternalInput")
    wq = nc.dram_tensor("wq", [C, 2 * D], F32R, kind="ExternalInput")
    wk = nc.dram_tensor("wk", [C, 2 * D], F32R, kind="ExternalInput")
    wv = nc.dram_tensor("wv", [C, 2 * D], F32R, kind="ExternalInput")
    y = nc.dram_tensor("y", [batches, t_len, 2 * D], F32, kind="ExternalOutput")

    with tile.TileContext(nc) as tc:
        with (
            tc.tile_pool(name="consts", bufs=1) as consts,
            tc.tile_pool(name="wpool", bufs=1) as wpool,
            tc.tile_pool(name="qkv", bufs=batches) as qkv,
            tc.tile_pool(name="epool", bufs=2) as epool,
            tc.tile_pool(name="avs", bufs=2) as avs,
            tc.tile_pool(name="outp", bufs=8) as outp,
            tc.tile_pool(name="small", bufs=8) as small,
        ):
            identity = consts.tile([128, 128], F32)
            make_identity(nc, identity)
            # trimask[s, t_local] = 0 where t_local >= s else MASK_VAL
            trimask = consts.tile([128, 128], F32)
            nc.gpsimd.memset(trimask, 0.0)
            nc.gpsimd.affine_select(
                out=trimask, in_=trimask,
                compare_op=mybir.AluOpType.is_ge,
                fill=MASK_VAL, base=0,
                pattern=[[1, 128]], channel_multiplier=-1,
            )

            w_sb = {}
            for name, w in (("q", wq), ("k", wk), ("v", wv)):
                wt = wpool.tile([128, NCH, 2 * D], F32R, tag=f"w{name}", name=f"w{name}_sb")
                nc.sync.dma_start(out=wt, in_=w.rearrange("(k p) d -> p k d", p=128))
                w_sb[name] = wt

            # Persistent per-batch tensors
            QT, KT, VH = {}, {}, {}
            for b in range(batches):
                QT[b] = qkv.tile([128, t_len], F32R, tag="qt", name=f"qt{b}")
                KT[b] = qkv.tile([128, t_len], F32R, tag="kt", name=f"kt{b}")
                for h in range(HPC):
                    vh = qkv.tile([128, (t_len // SB) * (D + 1)], F32R, tag=f"vh{h}", name=f"vh{b}_{h}")
                    ones_view = vh.rearrange("p (i c) -> p i c", c=D + 1)[:, :, D:D + 1]
                    nc.gpsimd.memset(ones_view.bitcast(F32), 1.0)
                    VH[(b, h)] = vh

            # ---------------- projection phase ----------------
            with (
                tc.tile_pool(name="xtp", bufs=3) as xtp,
                tc.tile_pool(name="vts", bufs=2) as vts,
                tc.tile_pool(name="ppsum", bufs=3, space="PSUM") as ppsum,
                tc.tile_pool(name="tpsum", bufs=2, space="PSUM") as tpsum,
            ):
                for b in range(batches):
                    xr = xt[b].rearrange("(k p) t -> p k t", p=128)
                    for j in range(nj):
                        xt_sb = xtp.tile([128, NCH, TT], F32R)
                        nc.sync.dma_start(
                            out=xt_sb, in_=xr[:, :, j * TT:(j + 1) * TT])
                        for name in ("q", "k", "v"):
                            pp = ppsum.tile([128, TT], F32, tag="proj")
                            for kk in range(NCH):
                                nc.tensor.matmul(
                                    pp,
                                    lhsT=w_sb[name][:, kk, :],
                                    rhs=xt_sb[:, kk, :],
                                    start=(kk == 0), stop=(kk == NCH - 1),
                                )
                            if name == "q":
                                nc.vector.tensor_copy(
                                    QT[b][:, j * TT:(j + 1) * TT], pp)
                            elif name == "k":
                                nc.vector.tensor_copy(
                                    KT[b][:, j * TT:(j + 1) * TT], pp)
                            else:
                                vt_sb = vts.tile([128, TT], F32)
                                nc.vector.tensor_copy(vt_sb, pp)
                                for q4 in range(TT // 128):
                                    vp = tpsum.tile([128, 128], F32)
                                    nc.tensor.transpose(
                                        vp, vt_sb[:, q4 * 128:(q4 + 1) * 128],
                                        identity)
                                    sb = (j * TT) // SB + q4
                                    for h in range(HPC):
                                        nc.vector.tensor_copy(
                                            VH[(b, h)][:, sb * (D + 1):sb * (D + 1) + D],
                                            vp[:, h * D:(h + 1) * D])

            # ---------------- attention phase ----------------
            with (
                tc.tile_pool(name="spsum", bufs=2, space="PSUM") as spsum,
                tc.tile_pool(name="avpsum", bufs=1, space="PSUM") as avpsum,
                tc.tile_pool(name="opsum", bufs=1, space="PSUM") as opsum,
            ):
                for b in range(batches):
                    for j in range(nj):
                        out_tiles = [outp.tile([128, 2 * D], F32, tag="out", name=f"out{b}_{j}_{q}")
                                     for q in range(TT // 128)]
                        for h in range(HPC):
                            hp = slice(h * D, (h + 1) * D)
                            n_sb = (j + 1) * TT // SB
                            av_ps = avpsum.tile([D + 1, TT], F32)
                            i = 0
                            while i < n_sb:
                                gw = min(GS, n_sb - i)
                                S = spsum.tile([128, GS * TT], F32, tag="spsum")
                                for m in range(gw):
                                    sb = i + m
                                    nc.tensor.matmul(
                                        S[:, m * TT:(m + 1) * TT],
                                        lhsT=KT[b][hp, sb * SB:(sb + 1) * SB],
                                        rhs=QT[b][hp, j * TT:(j + 1) * TT],
                                        start=True, stop=True,
                                    )
                                    if sb >= 4 * j:  # diagonal block
                                        mloc = sb - 4 * j
                                        off = m * TT + mloc * SB
                                        nc.vector.tensor_add(
                                            S[:, off:off + SB],
                                            S[:, off:off + SB], trimask)
                                eg = epool.tile([128, GS * TT], F32R, tag="e")
                                nc.scalar.activation(
                                    out=eg[:, :gw * TT], in_=S[:, :gw * TT],
                                    func=mybir.ActivationFunctionType.Exp)
                                for m in range(gw):
                                    sb = i + m
                                    if sb > 4 * j:
                                        mloc = sb - 4 * j
                                        nc.gpsimd.memset(
                                            eg[:, m * TT:m * TT + mloc * SB].bitcast(F32), 0.0)
                                for m in range(gw):
                                    sb = i + m
                                    nc.tensor.matmul(
                                        av_ps,
                                        lhsT=VH[(b, h)][:, sb * (D + 1):(sb + 1) * (D + 1)],
                                        rhs=eg[:, m * TT:(m + 1) * TT],
                                        start=(sb == 0), stop=(sb == n_sb - 1),
                                        skip_group_check=True,
                                    )
                                i += gw
                            av_sb = avs.tile([D + 1, TT], F32)
                            nc.vector.tensor_copy(av_sb, av_ps)
                            for q4 in range(TT // 128):
                                ot = opsum.tile([128, D + 1], F32)
                                nc.tensor.transpose(
                                    ot, av_sb[:, q4 * 128:(q4 + 1) * 128],
                                    identity[0:D + 1, 0:D + 1])
                                rec = small.tile([128, 1], F32)
                                nc.vector.reciprocal(rec, ot[:, D:D + 1])
                                nc.vector.tensor_scalar_mul(
                                    out_tiles[q4][:, h * D:(h + 1) * D],
                                    ot[:, 0:D], rec)
                        for q4 in range(TT // 128):
                            t0 = j * TT + q4 * 128
                            nc.sync.dma_start(
                                out=y[b, t0:t0 + 128, :], in_=out_tiles[q4])

    nc.compile()
    return nc


_CACHE = {}


def _get_runner():
    if "run" in _CACHE:
        return _CACHE["run"]

    import jax
    from jax.experimental.shard_map import shard_map
    from jax.sharding import Mesh, PartitionSpec
    from concourse import bass2jax
    from concourse.bass2jax import _bass_exec_p, install_neuronx_cc_hook

    nc = build_nc()
    install_neuronx_cc_hook()

    partition_name = (nc.partition_id_tensor.name
                      if nc.partition_id_tensor else None)
    in_names, out_names, out_avals, zero_outs = [], [], [], []
    for alloc in nc.m.functions[0].allocations:
        if not isinstance(alloc, mybir.MemoryLocationSet):
            continue
        name = alloc.memorylocations[0].name
        if alloc.kind == "ExternalInput":
            if name != partition_name:
                in_names.append(name)
        elif alloc.kind == "ExternalOutput":
            out_names.append(name)
            shape = tuple(alloc.tensor_shape)
            dtype = mybir.dt.np(alloc.dtype)
            out_avals.append(jax.core.ShapedArray(shape, dtype))
            zero_outs.append(np.zeros(shape, dtype))
    n_params = len(in_names)
    n_outs = len(out_avals)
    all_names = in_names + out_names
    if partition_name is not None:
        all_names = all_names + [partition_name]
    donate = tuple(range(n_params, n_params + n_outs))

    def _body(*args):
        operands = list(args)
        if partition_name is not None:
            operands.append(bass2jax.partition_id_tensor())
        outs = _bass_exec_p.bind(
            *operands,
            out_avals=tuple(out_avals),
            in_names=tuple(all_names),
            out_names=tuple(out_names),
            lowering_input_output_aliases=(),
            sim_require_finite=True,
            sim_require_nnan=True,
            nc=nc,
        )
        return tuple(outs)

    devices = jax.devices()[:NCORES]
    mesh = Mesh(np.asarray(devices), ("core",))
    in_specs = (PartitionSpec("core"),) * (n_params + n_outs)
    out_specs = (PartitionSpec("core"),) * n_outs
    sharded = jax.jit(
        shard_map(_body, mesh=mesh, in_specs=in_specs, out_specs=out_specs,
                  check_rep=False),
        donate_argnums=donate, keep_unused=True,
    )

    runner = {
        "sharded": sharded,
        "in_names": in_names,
        "out_names": out_names,
        "out_avals": out_avals,
        "zero_outs": zero_outs,
    }
    _CACHE["run"] = runner
    return runner


def _shard_inputs(x, Wq, Wk, Wv):
    """Per-core input dicts. Host-side layout prep only."""
    scale = float(C) ** -0.5
    xt = np.ascontiguousarray(np.transpose(x, (0, 2, 1)))  # [B, C, T]
    maps = []
    for c in range(NCORES):
        h0 = HPC * c
        wq2 = np.ascontiguousarray(
            np.concatenate([Wq[h0 + i] for i in range(HPC)], axis=1) * scale,
            dtype=np.float32)
        wk2 = np.ascontiguousarray(
            np.concatenate([Wk[h0 + i] for i in range(HPC)], axis=1),
            dtype=np.float32)
        wv2 = np.ascontiguousarray(
            np.concatenate([Wv[h0 + i] for i in range(HPC)], axis=1),
            dtype=np.float32)
        maps.append({"xt": xt, "wq": wq2, "wk": wk2, "wv": wv2})
    return maps


def run_sharded(in_maps):
    """Run the 8-core NEFF once; returns list of per-core output dicts."""
    r = _get_runner()
    concat_in = [
        np.concatenate([in_maps[c][name] for c in range(NCORES)], axis=0)
        for name in r["in_names"]
    ]
    concat_zeros = [
        np.zeros((NCORES * z.shape[0], *z.shape[1:]), z.dtype)
        for z in r["zero_outs"]
    ]
    out_arrs = r["sharded"](*concat_in, *concat_zeros)
    return [
        {
            name: np.asarray(out_arrs[i]).reshape(
                NCORES, *r["out_avals"][i].shape)[c]
            for i, name in enumerate(r["out_names"])
        }
        for c in range(NCORES)
    ]


def kernel(x, Wq, Wk, Wv):
    in_maps = _shard_inputs(
        np.asarray(x, dtype=np.float32), np.asarray(Wq, dtype=np.float32),
        np.asarray(Wk, dtype=np.float32), np.asarray(Wv, dtype=np.float32))
    results = run_sharded(in_maps)
    return np.concatenate([results[c]["y"] for c in range(NCORES)], axis=2)


# revision 26
# speedup vs baseline: 419.3352x; 419.3352x over previous
"""Multi-head causal attention (B=2, T=2048, C=1024, H=16, D=64) on 8 TRN2 cores.

Sharding: 2 heads per core (tensor-parallel over H). x is replicated (passed
pre-transposed as x^T so the contraction dim lands on SBUF partitions). Each
core computes y[:, :, 2c*64:(2c+2)*64]; host concatenates along channels.

Per-core dataflow (f32r matmuls everywhere except bf16 Q/K for scores;
f32r = full PE rate at N>=256 with ~11-bit-mantissa rounding):
  1. Projections, W stationary -> Q^T/K^T/V^T in [dd=2*64, t] layout (both
     heads stacked on partitions); scale 1/sqrt(C) folded into Wq on host.
     Q^T/K^T drain PSUM->SBUF as bf16, V^T as f32.
  2. V^T PE-transposed to V[s, d] per head with a ones column appended
     (V_aug[s, 65]) so the AV matmul also emits softmax sums for free.
  3. Scores S^T[s, t] = K^T(stationary) x Q^T(moving) per s-block, both
     heads paired in one 2-bank PSUM tile; columns below the causal
     diagonal are never computed (trimmed N).
  4. ONE exp call per s-block covers both heads PSUM->SBUF (f32r out, no
     max-subtraction needed: |scores| <= ~1); the diagonal 128x128 gets a
     multiplicative 0/1 triangle on DVE, off the ScalarE critical path.
  5. AV: V_aug stationary, E^T moving (N trimmed), accumulated over
     s-blocks in PSUM -> out^T[65, t] (row 64 = softmax sums).
  6. PE-transpose to [t, 65], DVE reciprocal of the sums column +
     per-partition scalar multiply, DMA out.

Schedule: one fused streaming pipeline per (b, t-tile); the NEXT tile's
projection work is emitted in closures interleaved between the current
tile's attention s-block periods, and AV lags scores by 2 s-blocks, so the
PE never idles (keeps the HAM clock gate at K=8/8) and ScalarE never
starves at tile boundaries.
"""

import numpy as np

import concourse.mybir as mybir
import concourse.tile as tile
from concourse import bacc
from concourse.masks import make_identity

B, T, C, H, D = 2, 2048, 1024, 16, 64
HPC = 2          # heads per core
NCORES = 8
TT = 512         # t-tile (moving free dim)
SB = 128         # s-block (scores stationary free dim)
NCH = C // 128   # contraction chunks for projections
GS = 2           # s-blocks per exp group
F32 = mybir.dt.float32
F32R = mybir.dt.float32r
BF16 = mybir.dt.bfloat16
MASK_VAL = -1e10


def build_nc(t_len=T, batches=B):
    nj = t_len // TT
    nc = bacc.Bacc("TRN2", target_bir_lowering=False, debug=False)
    xt = nc.dram_tensor("xt", [batches, C, t_len], F32R, kind="ExternalInput")
    wq = nc.dram_tensor("wq", [C, 2 * D], F32R, kind="ExternalInput")
    wk = nc.dram_tensor("wk", [C, 2 * D], F32R, kind="ExternalInput")
    wv = nc.dram_tensor("wv", [C, 2 * D], F32R, kind="ExternalInput")
    y = nc.dram_tensor("y", [batches, t_len, 2 * D], F32, kind="ExternalOutput")

    with tile.TileContext(nc) as tc:
        with (
            tc.tile_pool(name="consts", bufs=1) as consts,
            tc.tile_pool(name="wpool", bufs=1) as wpool,
            tc.tile_pool(name="qkv", bufs=batches) as qkv,
            tc.tile_pool(name="epool", bufs=4) as epool,
            tc.tile_pool(name="avs", bufs=2) as avs,
            tc.tile_pool(name="outp", bufs=8) as outp,
            tc.tile_pool(name="small", bufs=8) as small,
        ):
            identity = consts.tile([128, 128], F32)
            make_identity(nc, identity)
            # tri01[s, t_local] = 1 where t_local >= s else 0; multiplied
            # into the diagonal 128x128 sub-block of E after exp.
            tri01 = consts.tile([128, SB], F32R)
            nc.gpsimd.memset(tri01.bitcast(F32), 1.0)
            nc.gpsimd.affine_select(
                out=tri01.bitcast(F32), in_=tri01.bitcast(F32),
                compare_op=mybir.AluOpType.is_ge,
                fill=0.0, base=0,
                pattern=[[1, SB]], channel_multiplier=-1,
            )

            w_sb = {}
            for name, w in (("q", wq), ("k", wk), ("v", wv)):
                wt = wpool.tile([128, NCH, 2 * D], F32R, tag=f"w{name}", name=f"w{name}_sb")
                nc.sync.dma_start(out=wt, in_=w.rearrange("(k p) d -> p k d", p=128))
                w_sb[name] = wt

            # Persistent per-batch tensors
            QT, KT, VH = {}, {}, {}
            for b in range(batches):
                QT[b] = qkv.tile([128, t_len], BF16, tag="qt", name=f"qt{b}")
                KT[b] = qkv.tile([128, t_len], BF16, tag="kt", name=f"kt{b}")
                for h in range(HPC):
                    vh = qkv.tile([128, (t_len // SB) * (D + 1)], F32R, tag=f"vh{h}", name=f"vh{b}_{h}")
                    ones_view = vh.rearrange("p (i c) -> p i c", c=D + 1)[:, :, D:D + 1]
                    nc.gpsimd.memset(ones_view.bitcast(F32), 1.0)
                    VH[(b, h)] = vh

            # ---------------- fused streaming pipeline ----------------
            # Per (b, j): projections for t-tile j, then causal attention for
            # t-tile j (which only needs K/V up to tile j). One PSUM budget,
            # no phase boundary, so the PE stays continuously busy and the
            # HAM clock-gate stays warm. The attention inner loop software-
            # pipelines two head-streams with lag-1 AV so the PE never
            # stalls on exp.
            with (
                tc.tile_pool(name="xtp", bufs=3) as xtp,
                tc.tile_pool(name="vts", bufs=2) as vts,
                tc.tile_pool(name="mixps", bufs=2, space="PSUM") as mixps,
                tc.tile_pool(name="spsum", bufs=2, space="PSUM") as spsum,
                tc.tile_pool(name="avpsum", bufs=2, space="PSUM") as avpsum,
            ):
                def proj_closures(b, j):
                    """Projection work for (b, j) as a list of closures, to
                    be interleaved into the previous tile's attention
                    periods so neither PE nor ScalarE ever starves."""
                    state = {}

                    def do_load():
                        xr = xt[b].rearrange("(k p) t -> p k t", p=128)
                        xt_sb = xtp.tile([128, NCH, TT], F32R, tag="xts",
                                         name=f"xts{b}_{j}")
                        nc.sync.dma_start(
                            out=xt_sb, in_=xr[:, :, j * TT:(j + 1) * TT])
                        state["xt"] = xt_sb

                    def do_proj(name):
                        pp = mixps.tile([128, TT], F32, tag="mix",
                                        name=f"pp_{name}")
                        for kk in range(NCH):
                            nc.tensor.matmul(
                                pp,
                                lhsT=w_sb[name][:, kk, :],
                                rhs=state["xt"][:, kk, :],
                                start=(kk == 0), stop=(kk == NCH - 1),
                                skip_group_check=True,
                            )
                        if name == "q":
                            nc.vector.tensor_copy(
                                QT[b][:, j * TT:(j + 1) * TT], pp)
                        elif name == "k":
                            nc.vector.tensor_copy(
                                KT[b][:, j * TT:(j + 1) * TT], pp)
                        else:
                            vt_sb = vts.tile([128, TT], F32, tag="vt",
                                             name=f"vt{b}_{j}")
                            nc.vector.tensor_copy(vt_sb, pp)
                            state["vt"] = vt_sb

                    def do_vtrans(q4):
                        vp = mixps.tile([128, 128], F32, tag="mix",
                                        name=f"vp{q4}")
                        nc.tensor.transpose(
                            vp, state["vt"][:, q4 * 128:(q4 + 1) * 128],
                            identity)
                        sb = (j * TT) // SB + q4
                        for h in range(HPC):
                            nc.vector.tensor_copy(
                                VH[(b, h)][:, sb * (D + 1):sb * (D + 1) + D],
                                vp[:, h * D:(h + 1) * D])

                    ops = [lambda: (do_load(), do_proj("q"))[1],
                           lambda: do_proj("k"),
                           lambda: do_proj("v")]
                    ops += [lambda q4=q4: do_vtrans(q4)
                            for q4 in range(TT // 128)]
                    return ops

                def emit_attention(b, j, pending):
                    """Causal attention for t-tile j. Per s-block: both
                    heads' score MMs into one paired PSUM tile [h0 | h1]
                    (disjoint row groups -> concurrent), ONE exp call for
                    both heads, multiplicative tri-mask on E after exp (off
                    the ACT critical path), AV lagging 2 s-blocks. Closures
                    in `pending` (next tile's projections) are drained
                    evenly across the periods."""
                    out_tiles = [outp.tile([128, 2 * D], F32, tag="out",
                                           name=f"out{b}_{j}_{q}")
                                 for q in range(TT // 128)]
                    n_sb = (j + 1) * TT // SB
                    av_ps = {h: avpsum.tile([D + 1, TT], F32, tag="avps",
                                            name=f"avps{h}")
                             for h in range(HPC)}
                    eg = {}
                    LAG = 2

                    def emit_scores(sb):
                        # off: columns below the causal diagonal are never
                        # computed (scores, exp, AV all trimmed to t >= s).
                        off = max(0, (sb - 4 * j) * SB)
                        S = spsum.tile([128, HPC * TT], F32,
                                       tag="spsum", name=f"s{sb}")
                        for h in range(HPC):
                            hp = slice(h * D, (h + 1) * D)
                            nc.tensor.matmul(
                                S[:, h * TT + off:(h + 1) * TT],
                                lhsT=KT[b][hp, sb * SB:(sb + 1) * SB],
                                rhs=QT[b][hp, j * TT + off:(j + 1) * TT],
                                start=True, stop=True,
                            )
                        e = epool.tile([128, HPC * TT], F32R, tag="e",
                                       name=f"e{sb}")
                        if off == 0:
                            nc.scalar.activation(
                                out=e, in_=S,
                                func=mybir.ActivationFunctionType.Exp)
                        else:
                            for h in range(HPC):
                                nc.scalar.activation(
                                    out=e[:, h * TT + off:(h + 1) * TT],
                                    in_=S[:, h * TT + off:(h + 1) * TT],
                                    func=mybir.ActivationFunctionType.Exp)
                        if sb >= 4 * j:  # diagonal triangle at cols [off, off+SB)
                            for h in range(HPC):
                                nc.vector.tensor_mul(
                                    e[:, h * TT + off:h * TT + off + SB],
                                    e[:, h * TT + off:h * TT + off + SB],
                                    tri01)
                        eg[sb] = (e, off)

                    def emit_av(sb):
                        e, off = eg.pop(sb)
                        for h in range(HPC):
                            nc.tensor.matmul(
                                av_ps[h][:, off:],
                                lhsT=VH[(b, h)][:, sb * (D + 1):(sb + 1) * (D + 1)],
                                rhs=e[:, h * TT + off:(h + 1) * TT],
                                start=(sb == 0), stop=(sb == n_sb - 1),
                                skip_group_check=True,
                            )

                    n_periods = n_sb + LAG
                    n_pend = len(pending)
                    popped = 0
                    for sb in range(n_periods):
                        if sb < n_sb:
                            emit_scores(sb)
                        want = (n_pend * (sb + 1)) // n_periods
                        while popped < want:
                            pending[popped]()
                            popped += 1
                        if sb >= LAG:
                            emit_av(sb - LAG)
                    assert popped == n_pend

                    for h in range(HPC):
                        av_sb = avs.tile([D + 1, TT], F32, name=f"avsb{h}")
                        nc.vector.tensor_copy(av_sb, av_ps[h])
                        for q4 in range(TT // 128):
                            ot = mixps.tile([128, D + 1], F32, tag="mix",
                                            name=f"ot{h}_{q4}")
                            nc.tensor.transpose(
                                ot, av_sb[:, q4 * 128:(q4 + 1) * 128],
                                identity[0:D + 1, 0:D + 1])
                            rec = small.tile([128, 1], F32)
                            nc.vector.reciprocal(rec, ot[:, D:D + 1])
                            nc.vector.tensor_scalar_mul(
                                out_tiles[q4][:, h * D:(h + 1) * D],
                                ot[:, 0:D], rec)
                    for q4 in range(TT // 128):
                        t0 = j * TT + q4 * 128
                        nc.sync.dma_start(
                            out=y[b, t0:t0 + 128, :], in_=out_tiles[q4])

                seq = [(b, j) for b in range(batches) for j in range(nj)]
                for op in proj_closures(*seq[0]):
                    op()
                for idx, (b, j) in enumerate(seq):
                    nxt = proj_closures(*seq[idx + 1]) if idx + 1 < len(seq) else []
                    emit_attention(b, j, nxt)

    nc.compile()
    return nc


_CACHE = {}


def _get_runner():
    if "run" in _CACHE:
        return _CACHE["run"]

    import jax
    from jax.experimental.shard_map import shard_map
    from jax.sharding import Mesh, PartitionSpec
    from concourse import bass2jax
    from concourse.bass2jax import _bass_exec_p, install_neuronx_cc_hook

    nc = build_nc()
    install_neuronx_cc_hook()

    partition_name = (nc.partition_id_tensor.name
                      if nc.partition_id_tensor else None)
    in_names, out_names, out_avals, zero_outs = [], [], [], []
    for alloc in nc.m.functions[0].allocations:
        if not isinstance(alloc, mybir.MemoryLocationSet):
            continue
        name = alloc.memorylocations[0].name
        if alloc.kind == "ExternalInput":
            if name != partition_name:
                in_names.append(name)
        elif alloc.kind == "ExternalOutput":
            out_names.append(name)
            shape = tuple(alloc.tensor_shape)
            dtype = mybir.dt.np(alloc.dtype)
            out_avals.append(jax.core.ShapedArray(shape, dtype))
            zero_outs.append(np.zeros(shape, dtype))
    n_params = len(in_names)
    n_outs = len(out_avals)
    all_names = in_names + out_names
    if partition_name is not None:
        all_names = all_names + [partition_name]
    donate = tuple(range(n_params, n_params + n_outs))

    def _body(*args):
        operands = list(args)
        if partition_name is not None:
            operands.append(bass2jax.partition_id_tensor())
        outs = _bass_exec_p.bind(
            *operands,
            out_avals=tuple(out_avals),
            in_names=tuple(all_names),
            out_names=tuple(out_names),
            lowering_input_output_aliases=(),
            sim_require_finite=True,
            sim_require_nnan=True,
            nc=nc,
        )
        return tuple(outs)

    devices = jax.devices()[:NCORES]
    mesh = Mesh(np.asarray(devices), ("core",))
    in_specs = (PartitionSpec("core"),) * (n_params + n_outs)
    out_specs = (PartitionSpec("core"),) * n_outs
    sharded = jax.jit(
        shard_map(_body, mesh=mesh, in_specs=in_specs, out_specs=out_specs,
                  check_rep=False),
        donate_argnums=donate, keep_unused=True,
    )

    runner = {
        "nc": nc,
        "all_names": all_names,
        "sharded": sharded,
        "in_names": in_names,
        "out_names": out_names,
        "out_avals": out_avals,
        "zero_outs": zero_outs,
    }
    _CACHE["run"] = runner
    return runner


def _shard_inputs(x, Wq, Wk, Wv):
    """Per-core input dicts. Host-side layout prep only."""
    scale = float(C) ** -0.5
    xt = np.ascontiguousarray(
        np.transpose(x, (0, 2, 1)).astype(np.float32))  # [B, C, T]
    maps = []
    for c in range(NCORES):
        h0 = HPC * c
        wq2 = np.ascontiguousarray(
            (np.concatenate([Wq[h0 + i] for i in range(HPC)], axis=1)
             * scale).astype(np.float32))
        wk2 = np.ascontiguousarray(
            np.concatenate([Wk[h0 + i] for i in range(HPC)], axis=1).astype(np.float32))
        wv2 = np.ascontiguousarray(
            np.concatenate([Wv[h0 + i] for i in range(HPC)], axis=1).astype(np.float32))
        maps.append({"xt": xt, "wq": wq2, "wk": wk2, "wv": wv2})
    return maps


def run_sharded(in_maps):
    """Run the 8-core NEFF once; returns list of per-core output dicts."""
    r = _get_runner()
    concat_in = [
        np.concatenate([in_maps[c][name] for c in range(NCORES)], axis=0)
        for name in r["in_names"]
    ]
    concat_zeros = [
        np.zeros((NCORES * z.shape[0], *z.shape[1:]), z.dtype)
        for z in r["zero_outs"]
    ]
    out_arrs = r["sharded"](*concat_in, *concat_zeros)
    return [
        {
            name: np.asarray(out_arrs[i]).reshape(
                NCORES, *r["out_avals"][i].shape)[c]
            for i, name in enumerate(r["out_names"])
        }
        for c in range(NCORES)
    ]


def kernel(x, Wq, Wk, Wv):
    in_maps = _shard_inputs(
        np.asarray(x, dtype=np.float32), np.asarray(Wq, dtype=np.float32),
        np.asarray(Wk, dtype=np.float32), np.asarray(Wv, dtype=np.float32))
    results = run_sharded(in_maps)
    return np.concatenate([results[c]["y"] for c in range(NCORES)], axis=2)


# revision 29
# speedup vs baseline: 421.5278x; 1.0052x over previous
"""Multi-head causal attention (B=2, T=2048, C=1024, H=16, D=64) on 8 TRN2 cores.

Sharding: 2 heads per core (tensor-parallel over H). x is replicated (passed
pre-transposed as x^T so the contraction dim lands on SBUF partitions). Each
core computes y[:, :, 2c*64:(2c+2)*64]; host concatenates along channels.

Per-core dataflow (f32r matmuls everywhere except bf16 Q/K for scores;
f32r = full PE rate at N>=256 with ~11-bit-mantissa rounding):
  1. Projections, W stationary -> Q^T/K^T/V^T in [dd=2*64, t] layout (both
     heads stacked on partitions); scale 1/sqrt(C) folded into Wq on host.
     Q^T/K^T drain PSUM->SBUF as bf16, V^T as f32.
  2. V^T PE-transposed to V[s, d] per head with a ones column appended
     (V_aug[s, 65]) so the AV matmul also emits softmax sums for free.
  3. Scores S^T[s, t] = K^T(stationary) x Q^T(moving) per s-block, both
     heads paired in one 2-bank PSUM tile; columns below the causal
     diagonal are never computed (trimmed N).
  4. ONE exp call per s-block covers both heads PSUM->SBUF (f32r out, no
     max-subtraction needed: |scores| <= ~1); the diagonal 128x128 gets a
     multiplicative 0/1 triangle on DVE, off the ScalarE critical path.
  5. AV: V_aug stationary, E^T moving (N trimmed), accumulated over
     s-blocks in PSUM -> out^T[65, t] (row 64 = softmax sums).
  6. PE-transpose to [t, 65], DVE reciprocal of the sums column +
     per-partition scalar multiply, DMA out.

Schedule: one fused streaming pipeline per (b, t-tile); the NEXT tile's
projection work is emitted in closures interleaved between the current
tile's attention s-block periods, and AV lags scores by 2 s-blocks, so the
PE never idles (keeps the HAM clock gate at K=8/8) and ScalarE never
starves at tile boundaries.
"""

import numpy as np

import concourse.mybir as mybir
import concourse.tile as tile
from concourse import bacc
from concourse.masks import make_identity

B, T, C, H, D = 2, 2048, 1024, 16, 64
HPC = 2          # heads per core
NCORES = 8
TT = 512         # t-tile (moving free dim)
SB = 128         # s-block (scores stationary free dim)
NCH = C // 128   # contraction chunks for projections
GS = 2           # s-blocks per exp group
F32 = mybir.dt.float32
F32R = mybir.dt.float32r
BF16 = mybir.dt.bfloat16
MASK_VAL = -1e10


def build_nc(t_len=T, batches=B):
    nj = t_len // TT
    nc = bacc.Bacc("TRN2", target_bir_lowering=False, debug=False)
    xt = nc.dram_tensor("xt", [batches, C, t_len], F32R, kind="ExternalInput")
    wq = nc.dram_tensor("wq", [C, 2 * D], F32R, kind="ExternalInput")
    wk = nc.dram_tensor("wk", [C, 2 * D], F32R, kind="ExternalInput")
    wv = nc.dram_tensor("wv", [C, 2 * D], F32R, kind="ExternalInput")
    y = nc.dram_tensor("y", [batches, t_len, 2 * D], F32, kind="ExternalOutput")

    with tile.TileContext(nc) as tc:
        with (
            tc.tile_pool(name="consts", bufs=1) as consts,
            tc.tile_pool(name="wpool", bufs=1) as wpool,
            tc.tile_pool(name="qkv", bufs=batches) as qkv,
            tc.tile_pool(name="epool", bufs=4) as epool,
            tc.tile_pool(name="avs", bufs=2) as avs,
            tc.tile_pool(name="outp", bufs=8) as outp,
            tc.tile_pool(name="small", bufs=8) as small,
        ):
            identity = consts.tile([128, 128], F32)
            make_identity(nc, identity)
            # tri01[s, t_local] = 1 where t_local >= s else 0; multiplied
            # into the diagonal 128x128 sub-block of E after exp.
            tri01 = consts.tile([128, SB], F32R)
            nc.gpsimd.memset(tri01.bitcast(F32), 1.0)
            nc.gpsimd.affine_select(
                out=tri01.bitcast(F32), in_=tri01.bitcast(F32),
                compare_op=mybir.AluOpType.is_ge,
                fill=0.0, base=0,
                pattern=[[1, SB]], channel_multiplier=-1,
            )

            w_sb = {}
            for name, w in (("q", wq), ("k", wk), ("v", wv)):
                wt = wpool.tile([128, NCH, 2 * D], F32R, tag=f"w{name}", name=f"w{name}_sb")
                nc.sync.dma_start(out=wt, in_=w.rearrange("(k p) d -> p k d", p=128))
                w_sb[name] = wt

            # Persistent per-batch tensors
            QT, KT, VH = {}, {}, {}
            for b in range(batches):
                QT[b] = qkv.tile([128, t_len], BF16, tag="qt", name=f"qt{b}")
                KT[b] = qkv.tile([128, t_len], BF16, tag="kt", name=f"kt{b}")
                for h in range(HPC):
                    vh = qkv.tile([128, (t_len // SB) * (D + 1)], F32R, tag=f"vh{h}", name=f"vh{b}_{h}")
                    ones_view = vh.rearrange("p (i c) -> p i c", c=D + 1)[:, :, D:D + 1]
                    nc.gpsimd.memset(ones_view.bitcast(F32), 1.0)
                    VH[(b, h)] = vh

            # ---------------- fused streaming pipeline ----------------
            # Per (b, j): projections for t-tile j, then causal attention for
            # t-tile j (which only needs K/V up to tile j). One PSUM budget,
            # no phase boundary, so the PE stays continuously busy and the
            # HAM clock-gate stays warm. The attention inner loop software-
            # pipelines two head-streams with lag-1 AV so the PE never
            # stalls on exp.
            with (
                tc.tile_pool(name="xtp", bufs=3) as xtp,
                tc.tile_pool(name="vts", bufs=2) as vts,
                tc.tile_pool(name="mixps", bufs=2, space="PSUM") as mixps,
                tc.tile_pool(name="spsum", bufs=2, space="PSUM") as spsum,
                tc.tile_pool(name="avpsum", bufs=2, space="PSUM") as avpsum,
            ):
                def proj_closures(b, j):
                    """Projection work for (b, j) as a list of closures, to
                    be interleaved into the previous tile's attention
                    periods so neither PE nor ScalarE ever starves."""
                    state = {}

                    def do_load():
                        xr = xt[b].rearrange("(k p) t -> p k t", p=128)
                        xt_sb = xtp.tile([128, NCH, TT], F32R, tag="xts",
                                         name=f"xts{b}_{j}")
                        # two half-tile DMAs so the first proj matmuls only
                        # wait for the first half (cuts pipeline-fill)
                        half = NCH // 2
                        nc.sync.dma_start(
                            out=xt_sb[:, 0:half, :],
                            in_=xr[:, 0:half, j * TT:(j + 1) * TT])
                        nc.sync.dma_start(
                            out=xt_sb[:, half:, :],
                            in_=xr[:, half:, j * TT:(j + 1) * TT])
                        state["xt"] = xt_sb

                    def do_proj(name):
                        pp = mixps.tile([128, TT], F32, tag="mix",
                                        name=f"pp_{name}")
                        for kk in range(NCH):
                            nc.tensor.matmul(
                                pp,
                                lhsT=w_sb[name][:, kk, :],
                                rhs=state["xt"][:, kk, :],
                                start=(kk == 0), stop=(kk == NCH - 1),
                                skip_group_check=True,
                            )
                        if name == "q":
                            nc.vector.tensor_copy(
                                QT[b][:, j * TT:(j + 1) * TT], pp)
                        elif name == "k":
                            nc.vector.tensor_copy(
                                KT[b][:, j * TT:(j + 1) * TT], pp)
                        else:
                            vt_sb = vts.tile([128, TT], F32, tag="vt",
                                             name=f"vt{b}_{j}")
                            nc.vector.tensor_copy(vt_sb, pp)
                            state["vt"] = vt_sb

                    def do_vtrans(q4):
                        vp = mixps.tile([128, 128], F32, tag="mix",
                                        name=f"vp{q4}")
                        nc.tensor.transpose(
                            vp, state["vt"][:, q4 * 128:(q4 + 1) * 128],
                            identity)
                        sb = (j * TT) // SB + q4
                        for h in range(HPC):
                            nc.vector.tensor_copy(
                                VH[(b, h)][:, sb * (D + 1):sb * (D + 1) + D],
                                vp[:, h * D:(h + 1) * D])

                    ops = [lambda: (do_load(), do_proj("q"))[1],
                           lambda: do_proj("k"),
                           lambda: do_proj("v")]
                    ops += [lambda q4=q4: do_vtrans(q4)
                            for q4 in range(TT // 128)]
                    return ops

                def emit_attention(b, j, pending):
                    """Causal attention for t-tile j. Per s-block: both
                    heads' score MMs into one paired PSUM tile [h0 | h1]
                    (disjoint row groups -> concurrent), ONE exp call for
                    both heads, multiplicative tri-mask on E after exp (off
                    the ACT critical path), AV lagging 2 s-blocks. Closures
                    in `pending` (next tile's projections) are drained
                    evenly across the periods."""
                    out_tiles = [outp.tile([128, 2 * D], F32, tag="out",
                                           name=f"out{b}_{j}_{q}")
                                 for q in range(TT // 128)]
                    n_sb = (j + 1) * TT // SB
                    av_ps = {h: avpsum.tile([D + 1, TT], F32, tag="avps",
                                            name=f"avps{h}")
                             for h in range(HPC)}
                    eg = {}
                    LAG = 2

                    def emit_scores(sb):
                        # off: columns below the causal diagonal are never
                        # computed (scores, exp, AV all trimmed to t >= s).
                        off = max(0, (sb - 4 * j) * SB)
                        S = spsum.tile([128, HPC * TT], F32,
                                       tag="spsum", name=f"s{sb}")
                        for h in range(HPC):
                            hp = slice(h * D, (h + 1) * D)
                            nc.tensor.matmul(
                                S[:, h * TT + off:(h + 1) * TT],
                                lhsT=KT[b][hp, sb * SB:(sb + 1) * SB],
                                rhs=QT[b][hp, j * TT + off:(j + 1) * TT],
                                start=True, stop=True,
                            )
                        e = epool.tile([128, HPC * TT], F32R, tag="e",
                                       name=f"e{sb}")
                        if off == 0:
                            nc.scalar.activation(
                                out=e, in_=S,
                                func=mybir.ActivationFunctionType.Exp)
                        else:
                            for h in range(HPC):
                                nc.scalar.activation(
                                    out=e[:, h * TT + off:(h + 1) * TT],
                                    in_=S[:, h * TT + off:(h + 1) * TT],
                                    func=mybir.ActivationFunctionType.Exp)
                        if sb >= 4 * j:  # diagonal triangle at cols [off, off+SB)
                            for h in range(HPC):
                                nc.vector.tensor_mul(
                                    e[:, h * TT + off:h * TT + off + SB],
                                    e[:, h * TT + off:h * TT + off + SB],
                                    tri01)
                        eg[sb] = (e, off)

                    def emit_av(sb):
                        e, off = eg.pop(sb)
                        for h in range(HPC):
                            nc.tensor.matmul(
                                av_ps[h][:, off:],
                                lhsT=VH[(b, h)][:, sb * (D + 1):(sb + 1) * (D + 1)],
                                rhs=e[:, h * TT + off:(h + 1) * TT],
                                start=(sb == 0), stop=(sb == n_sb - 1),
                                skip_group_check=True,
                            )

                    n_periods = n_sb + LAG
                    n_pend = len(pending)
                    popped = 0
                    for sb in range(n_periods):
                        if sb < n_sb:
                            emit_scores(sb)
                        want = (n_pend * (sb + 1)) // n_periods
                        while popped < want:
                            pending[popped]()
                            popped += 1
                        if sb >= LAG:
                            emit_av(sb - LAG)
                    assert popped == n_pend

                    for h in range(HPC):
                        av_sb = avs.tile([D + 1, TT], F32, name=f"avsb{h}")
                        nc.vector.tensor_copy(av_sb, av_ps[h])
                        for q4 in range(TT // 128):
                            ot = mixps.tile([128, D + 1], F32, tag="mix",
                                            name=f"ot{h}_{q4}")
                            nc.tensor.transpose(
                                ot, av_sb[:, q4 * 128:(q4 + 1) * 128],
                                identity[0:D + 1, 0:D + 1])
                            rec = small.tile([128, 1], F32)
                            nc.vector.reciprocal(rec, ot[:, D:D + 1])
                            nc.vector.tensor_scalar_mul(
                                out_tiles[q4][:, h * D:(h + 1) * D],
                                ot[:, 0:D], rec)
                    for q4 in range(TT // 128):
                        t0 = j * TT + q4 * 128
                        nc.sync.dma_start(
                            out=y[b, t0:t0 + 128, :], in_=out_tiles[q4])

                seq = [(b, j) for b in range(batches) for j in range(nj)]
                for op in proj_closures(*seq[0]):
                    op()
                for idx, (b, j) in enumerate(seq):
                    nxt = proj_closures(*seq[idx + 1]) if idx + 1 < len(seq) else []
                    emit_attention(b, j, nxt)

    nc.compile()
    return nc


_CACHE = {}


def _get_runner():
    if "run" in _CACHE:
        return _CACHE["run"]

    import jax
    from jax.experimental.shard_map import shard_map
    from jax.sharding import Mesh, PartitionSpec
    from concourse import bass2jax
    from concourse.bass2jax import _bass_exec_p, install_neuronx_cc_hook

    nc = build_nc()
    install_neuronx_cc_hook()

    partition_name = (nc.partition_id_tensor.name
                      if nc.partition_id_tensor else None)
    in_names, out_names, out_avals, zero_outs = [], [], [], []
    for alloc in nc.m.functions[0].allocations:
        if not isinstance(alloc, mybir.MemoryLocationSet):
            continue
        name = alloc.memorylocations[0].name
        if alloc.kind == "ExternalInput":
            if name != partition_name:
                in_names.append(name)
        elif alloc.kind == "ExternalOutput":
            out_names.append(name)
            shape = tuple(alloc.tensor_shape)
            dtype = mybir.dt.np(alloc.dtype)
            out_avals.append(jax.core.ShapedArray(shape, dtype))
            zero_outs.append(np.zeros(shape, dtype))
    n_params = len(in_names)
    n_outs = len(out_avals)
    all_names = in_names + out_names
    if partition_name is not None:
        all_names = all_names + [partition_name]
    donate = tuple(range(n_params, n_params + n_outs))

    def _body(*args):
        operands = list(args)
        if partition_name is not None:
            operands.append(bass2jax.partition_id_tensor())
        outs = _bass_exec_p.bind(
            *operands,
            out_avals=tuple(out_avals),
            in_names=tuple(all_names),
            out_names=tuple(out_names),
            lowering_input_output_aliases=(),
            sim_require_finite=True,
            sim_require_nnan=True,
            nc=nc,
        )
        return tuple(outs)

    devices = jax.devices()[:NCORES]
    mesh = Mesh(np.asarray(devices), ("core",))
    in_specs = (PartitionSpec("core"),) * (n_params + n_outs)
    out_specs = (PartitionSpec("core"),) * n_outs
    sharded = jax.jit(
        shard_map(_body, mesh=mesh, in_specs=in_specs, out_specs=out_specs,
                  check_rep=False),
        donate_argnums=donate, keep_unused=True,
    )

    runner = {
        "nc": nc,
        "all_names": all_names,
        "sharded": sharded,
        "in_names": in_names,
        "out_names": out_names,
        "out_avals": out_avals,
        "zero_outs": zero_outs,
    }
    _CACHE["run"] = runner
    return runner


def _shard_inputs(x, Wq, Wk, Wv):
    """Per-core input dicts. Host-side layout prep only."""
    scale = float(C) ** -0.5
    xt = np.ascontiguousarray(
        np.transpose(x, (0, 2, 1)).astype(np.float32))  # [B, C, T]
    maps = []
    for c in range(NCORES):
        h0 = HPC * c
        wq2 = np.ascontiguousarray(
            (np.concatenate([Wq[h0 + i] for i in range(HPC)], axis=1)
             * scale).astype(np.float32))
        wk2 = np.ascontiguousarray(
            np.concatenate([Wk[h0 + i] for i in range(HPC)], axis=1).astype(np.float32))
        wv2 = np.ascontiguousarray(
            np.concatenate([Wv[h0 + i] for i in range(HPC)], axis=1).astype(np.float32))
        maps.append({"xt": xt, "wq": wq2, "wk": wk2, "wv": wv2})
    return maps


def run_sharded(in_maps):
    """Run the 8-core NEFF once; returns list of per-core output dicts."""
    r = _get_runner()
    concat_in = [
        np.concatenate([in_maps[c][name] for c in range(NCORES)], axis=0)
        for name in r["in_names"]
    ]
    concat_zeros = [
        np.zeros((NCORES * z.shape[0], *z.shape[1:]), z.dtype)
        for z in r["zero_outs"]
    ]
    out_arrs = r["sharded"](*concat_in, *concat_zeros)
    return [
        {
            name: np.asarray(out_arrs[i]).reshape(
                NCORES, *r["out_avals"][i].shape)[c]
            for i, name in enumerate(r["out_names"])
        }
        for c in range(NCORES)
    ]


def kernel(x, Wq, Wk, Wv):
    in_maps = _shard_inputs(
        np.asarray(x, dtype=np.float32), np.asarray(Wq, dtype=np.float32),
        np.asarray(Wk, dtype=np.float32), np.asarray(Wv, dtype=np.float32))
    results = run_sharded(in_maps)
    return np.concatenate([results[c]["y"] for c in range(NCORES)], axis=2)


# revision 30
# speedup vs baseline: 426.9238x; 1.0128x over previous
"""Multi-head causal attention (B=2, T=2048, C=1024, H=16, D=64) on 8 TRN2 cores.

Sharding: 2 heads per core (tensor-parallel over H). x is replicated (passed
pre-transposed as x^T so the contraction dim lands on SBUF partitions). Each
core computes y[:, :, 2c*64:(2c+2)*64]; host concatenates along channels.

Per-core dataflow (f32r matmuls everywhere except bf16 Q/K for scores;
f32r = full PE rate at N>=256 with ~11-bit-mantissa rounding):
  1. Projections, W stationary -> Q^T/K^T/V^T in [dd=2*64, t] layout (both
     heads stacked on partitions); scale 1/sqrt(C) folded into Wq on host.
     Q^T/K^T drain PSUM->SBUF as bf16, V^T as f32.
  2. V^T PE-transposed to V[s, d] per head with a ones column appended
     (V_aug[s, 65]) so the AV matmul also emits softmax sums for free.
  3. Scores S^T[s, t] = K^T(stationary) x Q^T(moving) per s-block, both
     heads paired in one 2-bank PSUM tile; columns below the causal
     diagonal are never computed (trimmed N).
  4. ONE exp call per s-block covers both heads PSUM->SBUF (f32r out, no
     max-subtraction needed: |scores| <= ~1); the diagonal 128x128 gets a
     multiplicative 0/1 triangle on DVE, off the ScalarE critical path.
  5. AV: V_aug stationary, E^T moving (N trimmed), accumulated over
     s-blocks in PSUM -> out^T[65, t] (row 64 = softmax sums).
  6. PE-transpose to [t, 65], DVE reciprocal of the sums column +
     per-partition scalar multiply, DMA out.

Schedule: one fused streaming pipeline per (b, t-tile); the NEXT tile's
projection work is emitted in closures interleaved between the current
tile's attention s-block periods, and AV lags scores by 2 s-blocks, so the
PE never idles (keeps the HAM clock gate at K=8/8) and ScalarE never
starves at tile boundaries.
"""

import numpy as np

import concourse.mybir as mybir
import concourse.tile as tile
from concourse import bacc
from concourse.masks import make_identity

B, T, C, H, D = 2, 2048, 1024, 16, 64
HPC = 2          # heads per core
NCORES = 8
TT = 512         # t-tile (moving free dim)
SB = 128         # s-block (scores stationary free dim)
NCH = C // 128   # contraction chunks for projections
F32 = mybir.dt.float32
F32R = mybir.dt.float32r
BF16 = mybir.dt.bfloat16


def build_nc(t_len=T, batches=B):
    nj = t_len // TT
    nc = bacc.Bacc("TRN2", target_bir_lowering=False, debug=False)
    xt = nc.dram_tensor("xt", [batches, C, t_len], F32R, kind="ExternalInput")
    wq = nc.dram_tensor("wq", [C, 2 * D], F32R, kind="ExternalInput")
    wk = nc.dram_tensor("wk", [C, 2 * D], F32R, kind="ExternalInput")
    wv = nc.dram_tensor("wv", [C, 2 * D], F32R, kind="ExternalInput")
    y = nc.dram_tensor("y", [batches, t_len, 2 * D], F32, kind="ExternalOutput")

    with tile.TileContext(nc) as tc:
        with (
            tc.tile_pool(name="consts", bufs=1) as consts,
            tc.tile_pool(name="wpool", bufs=1) as wpool,
            tc.tile_pool(name="qkv", bufs=batches) as qkv,
            tc.tile_pool(name="epool", bufs=4) as epool,
            tc.tile_pool(name="avs", bufs=2) as avs,
            tc.tile_pool(name="outp", bufs=8) as outp,
            tc.tile_pool(name="small", bufs=8) as small,
        ):
            identity = consts.tile([128, 128], F32)
            make_identity(nc, identity)
            # tri01[s, t_local] = 1 where t_local >= s else 0; multiplied
            # into the diagonal 128x128 sub-block of E after exp.
            tri01 = consts.tile([128, SB], F32R)
            nc.gpsimd.memset(tri01.bitcast(F32), 1.0)
            nc.gpsimd.affine_select(
                out=tri01.bitcast(F32), in_=tri01.bitcast(F32),
                compare_op=mybir.AluOpType.is_ge,
                fill=0.0, base=0,
                pattern=[[1, SB]], channel_multiplier=-1,
            )

            w_sb = {}
            for name, w in (("q", wq), ("k", wk), ("v", wv)):
                wt = wpool.tile([128, NCH, 2 * D], F32R, tag=f"w{name}", name=f"w{name}_sb")
                nc.sync.dma_start(out=wt, in_=w.rearrange("(k p) d -> p k d", p=128))
                w_sb[name] = wt

            # Persistent per-batch tensors
            QT, KT, VH = {}, {}, {}
            for b in range(batches):
                QT[b] = qkv.tile([128, t_len], BF16, tag="qt", name=f"qt{b}")
                KT[b] = qkv.tile([128, t_len], BF16, tag="kt", name=f"kt{b}")
                for h in range(HPC):
                    vh = qkv.tile([128, (t_len // SB) * (D + 1)], F32R, tag=f"vh{h}", name=f"vh{b}_{h}")
                    ones_view = vh.rearrange("p (i c) -> p i c", c=D + 1)[:, :, D:D + 1]
                    nc.gpsimd.memset(ones_view.bitcast(F32), 1.0)
                    VH[(b, h)] = vh

            # ---------------- fused streaming pipeline ----------------
            # Per (b, j): projections for t-tile j, then causal attention for
            # t-tile j (which only needs K/V up to tile j). One PSUM budget,
            # no phase boundary, so the PE stays continuously busy and the
            # HAM clock-gate stays warm. The attention inner loop software-
            # pipelines two head-streams with lag-1 AV so the PE never
            # stalls on exp.
            with (
                tc.tile_pool(name="xtp", bufs=3) as xtp,
                tc.tile_pool(name="vts", bufs=2) as vts,
                tc.tile_pool(name="mixps", bufs=2, space="PSUM") as mixps,
                tc.tile_pool(name="spsum", bufs=2, space="PSUM") as spsum,
                tc.tile_pool(name="avpsum", bufs=2, space="PSUM") as avpsum,
            ):
                def proj_closures(b, j):
                    """Projection work for (b, j) as a list of closures, to
                    be interleaved into the previous tile's attention
                    periods so neither PE nor ScalarE ever starves."""
                    state = {}

                    def do_load():
                        xr = xt[b].rearrange("(k p) t -> p k t", p=128)
                        xt_sb = xtp.tile([128, NCH, TT], F32R, tag="xts",
                                         name=f"xts{b}_{j}")
                        # two half-tile DMAs so the first proj matmuls only
                        # wait for the first half (cuts pipeline-fill)
                        half = NCH // 2
                        nc.sync.dma_start(
                            out=xt_sb[:, 0:half, :],
                            in_=xr[:, 0:half, j * TT:(j + 1) * TT])
                        nc.sync.dma_start(
                            out=xt_sb[:, half:, :],
                            in_=xr[:, half:, j * TT:(j + 1) * TT])
                        state["xt"] = xt_sb

                    def do_proj(name):
                        pp = mixps.tile([128, TT], F32, tag="mix",
                                        name=f"pp_{name}")
                        for kk in range(NCH):
                            nc.tensor.matmul(
                                pp,
                                lhsT=w_sb[name][:, kk, :],
                                rhs=state["xt"][:, kk, :],
                                start=(kk == 0), stop=(kk == NCH - 1),
                                skip_group_check=True,
                            )
                        if name == "q":
                            nc.vector.tensor_copy(
                                QT[b][:, j * TT:(j + 1) * TT], pp)
                        elif name == "k":
                            nc.vector.tensor_copy(
                                KT[b][:, j * TT:(j + 1) * TT], pp)
                        else:
                            vt_sb = vts.tile([128, TT], F32, tag="vt",
                                             name=f"vt{b}_{j}")
                            nc.vector.tensor_copy(vt_sb, pp)
                            state["vt"] = vt_sb

                    def do_vtrans(q4):
                        vp = mixps.tile([128, 128], F32, tag="mix",
                                        name=f"vp{q4}")
                        nc.tensor.transpose(
                            vp, state["vt"][:, q4 * 128:(q4 + 1) * 128],
                            identity)
                        sb = (j * TT) // SB + q4
                        for h in range(HPC):
                            nc.vector.tensor_copy(
                                VH[(b, h)][:, sb * (D + 1):sb * (D + 1) + D],
                                vp[:, h * D:(h + 1) * D])

                    ops = [lambda: (do_load(), do_proj("q"))[1],
                           lambda: do_proj("k"),
                           lambda: do_proj("v")]
                    ops += [lambda q4=q4: do_vtrans(q4)
                            for q4 in range(TT // 128)]
                    return ops

                def emit_attention(b, j, pending):
                    """Causal attention for t-tile j. Per s-block: both
                    heads' score MMs into one paired PSUM tile [h0 | h1]
                    (disjoint row groups -> concurrent), ONE exp call for
                    both heads, multiplicative tri-mask on E after exp (off
                    the ACT critical path), AV lagging 2 s-blocks. Closures
                    in `pending` (next tile's projections) are drained
                    evenly across the periods."""
                    out_tiles = [outp.tile([128, 2 * D], F32, tag="out",
                                           name=f"out{b}_{j}_{q}")
                                 for q in range(TT // 128)]
                    n_sb = (j + 1) * TT // SB
                    av_ps = {h: avpsum.tile([D + 1, TT], F32, tag="avps",
                                            name=f"avps{h}")
                             for h in range(HPC)}
                    eg = {}
                    LAG = 2

                    def emit_scores(sb):
                        # off: columns below the causal diagonal are never
                        # computed (scores, exp, AV all trimmed to t >= s).
                        off = max(0, (sb - 4 * j) * SB)
                        S = spsum.tile([128, HPC * TT], F32,
                                       tag="spsum", name=f"s{sb}")
                        for h in range(HPC):
                            hp = slice(h * D, (h + 1) * D)
                            nc.tensor.matmul(
                                S[:, h * TT + off:(h + 1) * TT],
                                lhsT=KT[b][hp, sb * SB:(sb + 1) * SB],
                                rhs=QT[b][hp, j * TT + off:(j + 1) * TT],
                                start=True, stop=True,
                            )
                        e = epool.tile([128, HPC * TT], F32R, tag="e",
                                       name=f"e{sb}")
                        if off == 0:
                            nc.scalar.activation(
                                out=e, in_=S,
                                func=mybir.ActivationFunctionType.Exp)
                        else:
                            for h in range(HPC):
                                nc.scalar.activation(
                                    out=e[:, h * TT + off:(h + 1) * TT],
                                    in_=S[:, h * TT + off:(h + 1) * TT],
                                    func=mybir.ActivationFunctionType.Exp)
                        if sb >= 4 * j:  # diagonal triangle at cols [off, off+SB)
                            for h in range(HPC):
                                nc.vector.tensor_mul(
                                    e[:, h * TT + off:h * TT + off + SB],
                                    e[:, h * TT + off:h * TT + off + SB],
                                    tri01)
                        eg[sb] = (e, off)

                    def emit_av(sb):
                        e, off = eg.pop(sb)
                        for h in range(HPC):
                            nc.tensor.matmul(
                                av_ps[h][:, off:],
                                lhsT=VH[(b, h)][:, sb * (D + 1):(sb + 1) * (D + 1)],
                                rhs=e[:, h * TT + off:(h + 1) * TT],
                                start=(sb == 0), stop=(sb == n_sb - 1),
                                skip_group_check=True,
                            )

                    n_periods = n_sb + LAG
                    n_pend = len(pending)
                    popped = 0
                    for sb in range(n_periods):
                        if sb < n_sb:
                            emit_scores(sb)
                        want = (n_pend * (sb + 1)) // n_periods
                        while popped < want:
                            pending[popped]()
                            popped += 1
                        if sb >= LAG:
                            emit_av(sb - LAG)
                    assert popped == n_pend

                    for h in range(HPC):
                        av_sb = avs.tile([D + 1, TT], F32, name=f"avsb{h}")
                        nc.vector.tensor_copy(av_sb, av_ps[h])
                        for q4 in range(TT // 128):
                            ot = mixps.tile([128, D + 1], F32, tag="mix",
                                            name=f"ot{h}_{q4}")
                            nc.tensor.transpose(
                                ot, av_sb[:, q4 * 128:(q4 + 1) * 128],
                                identity[0:D + 1, 0:D + 1])
                            rec = small.tile([128, 1], F32)
                            nc.vector.reciprocal(rec, ot[:, D:D + 1])
                            nc.vector.tensor_scalar_mul(
                                out_tiles[q4][:, h * D:(h + 1) * D],
                                ot[:, 0:D], rec)
                    for q4 in range(TT // 128):
                        t0 = j * TT + q4 * 128
                        nc.sync.dma_start(
                            out=y[b, t0:t0 + 128, :], in_=out_tiles[q4])

                seq = [(b, j) for b in range(batches) for j in range(nj)]
                for op in proj_closures(*seq[0]):
                    op()
                for idx, (b, j) in enumerate(seq):
                    nxt = proj_closures(*seq[idx + 1]) if idx + 1 < len(seq) else []
                    emit_attention(b, j, nxt)

    nc.compile()
    return nc


_CACHE = {}


def _get_runner():
    if "run" in _CACHE:
        return _CACHE["run"]

    import jax
    from jax.experimental.shard_map import shard_map
    from jax.sharding import Mesh, PartitionSpec
    from concourse import bass2jax
    from concourse.bass2jax import _bass_exec_p, install_neuronx_cc_hook

    nc = build_nc()
    install_neuronx_cc_hook()

    partition_name = (nc.partition_id_tensor.name
                      if nc.partition_id_tensor else None)
    in_names, out_names, out_avals, zero_outs = [], [], [], []
    for alloc in nc.m.functions[0].allocations:
        if not isinstance(alloc, mybir.MemoryLocationSet):
            continue
        name = alloc.memorylocations[0].name
        if alloc.kind == "ExternalInput":
            if name != partition_name:
                in_names.append(name)
        elif alloc.kind == "ExternalOutput":
            out_names.append(name)
            shape = tuple(alloc.tensor_shape)
            dtype = mybir.dt.np(alloc.dtype)
            out_avals.append(jax.core.ShapedArray(shape, dtype))
            zero_outs.append(np.zeros(shape, dtype))
    n_params = len(in_names)
    n_outs = len(out_avals)
    all_names = in_names + out_names
    if partition_name is not None:
        all_names = all_names + [partition_name]
    donate = tuple(range(n_params, n_params + n_outs))

    def _body(*args):
        operands = list(args)
        if partition_name is not None:
            operands.append(bass2jax.partition_id_tensor())
        outs = _bass_exec_p.bind(
            *operands,
            out_avals=tuple(out_avals),
            in_names=tuple(all_names),
            out_names=tuple(out_names),
            lowering_input_output_aliases=(),
            sim_require_finite=True,
            sim_require_nnan=True,
            nc=nc,
        )
        return tuple(outs)

    devices = jax.devices()[:NCORES]
    mesh = Mesh(np.asarray(devices), ("core",))
    in_specs = (PartitionSpec("core"),) * (n_params + n_outs)
    out_specs = (PartitionSpec("core"),) * n_outs
    sharded = jax.jit(
        shard_map(_body, mesh=mesh, in_specs=in_specs, out_specs=out_specs,
                  check_rep=False),
        donate_argnums=donate, keep_unused=True,
    )

    runner = {
        "nc": nc,
        "all_names": all_names,
        "sharded": sharded,
        "in_names": in_names,
        "out_names": out_names,
        "out_avals": out_avals,
        "zero_outs": zero_outs,
    }
    _CACHE["run"] = runner
    return runner


def _shard_inputs(x, Wq, Wk, Wv):
    """Per-core input dicts. Host-side layout prep only."""
    scale = float(C) ** -0.5
    xt = np.ascontiguousarray(
        np.transpose(x, (0, 2, 1)).astype(np.float32))  # [B, C, T]
    maps = []
    for c in range(NCORES):
        h0 = HPC * c
        wq2 = np.ascontiguousarray(
            (np.concatenate([Wq[h0 + i] for i in range(HPC)], axis=1)
             * scale).astype(np.float32))
        wk2 = np.ascontiguousarray(
            np.concatenate([Wk[h0 + i] for i in range(HPC)], axis=1).astype(np.float32))
        wv2 = np.ascontiguousarray(
            np.concatenate([Wv[h0 + i] for i in range(HPC)], axis=1).astype(np.float32))
        maps.append({"xt": xt, "wq": wq2, "wk": wk2, "wv": wv2})
    return maps


def run_sharded(in_maps):
    """Run the 8-core NEFF once; returns list of per-core output dicts."""
    r = _get_runner()
    concat_in = [
        np.concatenate([in_maps[c][name] for c in range(NCORES)], axis=0)
        for name in r["in_names"]
    ]
    concat_zeros = [
        np.zeros((NCORES * z.shape[0], *z.shape[1:]), z.dtype)
        for z in r["zero_outs"]
    ]
    out_arrs = r["sharded"](*concat_in, *concat_zeros)
    return [
        {
            name: np.asarray(out_arrs[i]).reshape(
                NCORES, *r["out_avals"][i].shape)[c]
            for i, name in enumerate(r["out_names"])
        }
        for c in range(NCORES)
    ]


def kernel(x, Wq, Wk, Wv):
    in_maps = _shard_inputs(
        np.asarray(x, dtype=np.float32), np.asarray(Wq, dtype=np.float32),
        np.asarray(Wk, dtype=np.float32), np.asarray(Wv, dtype=np.float32))
    results = run_sharded(in_maps)
    return np.concatenate([results[c]["y"] for c in range(NCORES)], axis=2)
